# revision 2
# baseline (speedup 1.0000x reference)
"""GAT head (DGAT) Trainium2 kernel: 8-core row-sharded masked-softmax attention.

Math (per reference):
  h = X @ W                       [N, 64]
  e = leaky_relu(src_i + dst_j, 0.2), src = h@a[:64], dst = h@a[64:]
  att = softmax(where(adj>0, e, -9e15), axis=1)
  out = elu(att @ h)

Kernel strategy per core c (rows R = [1024c, 1024c+1024)):
  - Host pre-transposes the adjacency slab to [N, R] int8 with values
    (adj-1)*100 in {-100, 0}.
  - Per j-chunk [128j x 1024i]:
      e = leaky(src_i + dst_j)        (ACT Prelu or DVE 3-op, fp16)
      e += mask                       (SWDGE DMA int8->fp16 cast, CCE add:
                                       -100 on masked entries, 0 otherwise)
      p = exp(e)                      (ACT; masked entries underflow to 0)
      agg[65, i] += hext_j^T @ p      (PE; hext = [h | 1], row 64 = denom)
  - finalize: transpose agg back, scale by 1/denom, ELU, DMA out.
"""
import os
import sys
import numpy as np

sys.path.insert(0, "/opt/trn_rl_repo")

import concourse.bass as bass
import concourse.bacc as bacc
import concourse.tile as tile
from concourse import mybir
from concourse.masks import make_identity
from concourse import bass_utils

P = 128
N = 8192
DIN = 256
DOUT = 64
NCORES = 8
R = N // NCORES          # rows per core
MASKV = 100.0            # host bakes (adj-1)*MASKV into the int8 slab
ALPHA = 0.2
NJC = N // P             # 64 j-chunks
F32 = mybir.dt.float32
F16 = mybir.dt.float16
BF16 = mybir.dt.bfloat16
I8 = mybir.dt.int8

_cached = {}
ACT_PRELU_EVERY = 8      # jc % K == 0 -> prelu on ACT, else DVE leaky
AGG_DELAY = 1
E_BUFS = 4
PBUFS = 6


def build_module(rep=1, rep_loop=1):
    key = ("nc", rep, rep_loop)
    if key in _cached:
        return _cached[key]
    nc = bacc.Bacc("TRN2", target_bir_lowering=False, debug=False, num_devices=NCORES)

    adjm_d = nc.dram_tensor("adjm", [N, R], I8, kind="ExternalInput").ap()
    x_d = nc.dram_tensor("xt", [DIN, N], F32, kind="ExternalInput").ap()
    w_d = nc.dram_tensor("w", [DIN, DOUT], F32, kind="ExternalInput").ap()
    a_d = nc.dram_tensor("av", [2 * DOUT, 1], F32, kind="ExternalInput").ap()
    out_d = nc.dram_tensor("out", [R, DOUT], F32, kind="ExternalOutput").ap()

    with tile.TileContext(nc) as tc:
        for _ in range(rep):
            _build(nc, tc, adjm_d, x_d, w_d, a_d, out_d, rep_loop)

    nc.compile()
    _cached[key] = nc
    return nc


def _build(nc, tc, adjm_d, x_d, w_d, a_d, out_d, rep_loop=1):
    from contextlib import ExitStack

    with ExitStack() as ctx:
        const = ctx.enter_context(tc.tile_pool(name="const", bufs=1))

        # ---- constants ----
        idf = const.tile([P, P], F32)
        make_identity(nc, idf)
        ones1 = const.tile([1, P], F32)
        nc.vector.memset(ones1, 1.0)
        alpha_t = const.tile([P, 1], F32)
        nc.vector.memset(alpha_t, ALPHA)

        w_a = const.tile([P, DOUT], F32)
        w_b = const.tile([P, DOUT], F32)
        nc.sync.dma_start(out=w_a, in_=w_d[0:P, :])
        nc.sync.dma_start(out=w_b, in_=w_d[P:DIN, :])
        a1 = const.tile([DOUT, 1], F32)
        a2 = const.tile([DOUT, 1], F32)
        nc.sync.dma_start(out=a1, in_=a_d[0:DOUT, :])
        nc.sync.dma_start(out=a2, in_=a_d[DOUT : 2 * DOUT, :])

        # persistent per-core data
        hext_c = [const.tile([P, 65], BF16, tag=f"hx{c}", name=f"hx{c}")
                  for c in range(NJC)]          # [j%P, f|one] per j-chunk
        dstb8 = [const.tile([P, 8], F32, tag=f"db{b}", name=f"db{b}")
                 for b in range(NJC // 8)]      # dst, batches of 8 chunks
        src_my = const.tile([1, R], F32)
        s16 = const.tile([P, R], F16)           # src broadcast to 128 parts

        # ---- stage A: h = X@W (from pre-transposed X), hT, src, dst ----
        XCH = 4                 # x chunk tiles per half
        XW = N // XCH           # 2048 cols per chunk
        with tc.tile_pool(name="prep", bufs=1) as prep, \
             tc.tile_pool(name="prep_ps", bufs=2, space="PSUM") as prep_ps:
            xt_t = [prep.tile([P, XW], F32, tag=f"xta{k}", name=f"xta{k}")
                    for k in range(XCH)]
            xt_b = [prep.tile([P, XW], F32, tag=f"xtb{k}", name=f"xtb{k}")
                    for k in range(XCH)]
            ht_sb = prep.tile([DOUT, N], F32)   # h^T

            pid = nc.partition_id()

            for k in range(XCH):
                nc.sync.dma_start(out=xt_t[k], in_=x_d[0:P, k * XW : (k + 1) * XW])
                nc.sync.dma_start(out=xt_b[k], in_=x_d[P:DIN, k * XW : (k + 1) * XW])
            # hT [64, N] via f32r (1 cyc/col at >=256 free), dst batches
            # interleaved so dstb8[b] unblocks stage B as early as possible.
            for m in range(N // 512):
                k, off = m // (XW // 512), (m % (XW // 512)) * 512
                ht_ps = prep_ps.tile([DOUT, 512], F32, tag="pp", name="ht_ps")
                nc.tensor.matmul(ht_ps, lhsT=w_a,
                                 rhs=xt_t[k][:, off : off + 512],
                                 start=True, stop=False)
                nc.tensor.matmul(ht_ps, lhsT=w_b,
                                 rhs=xt_b[k][:, off : off + 512],
                                 start=False, stop=True)
                if m % 2 == 0:
                    nc.vector.tensor_copy(ht_sb[:, m * 512 : (m + 1) * 512], ht_ps)
                else:
                    nc.scalar.copy(ht_sb[:, m * 512 : (m + 1) * 512], ht_ps)
                if m % 2 == 1:
                    b = (m - 1) // 2
                    d_ps = prep_ps.tile([P, 8], F32, tag="pp", name="d_ps")
                    for bb in range(8):
                        c = b * 8 + bb
                        nc.tensor.matmul(d_ps[:, bb : bb + 1],
                                         lhsT=ht_sb[:, c * P : (c + 1) * P], rhs=a2,
                                         start=True, stop=True)
                    nc.vector.tensor_copy(dstb8[b], d_ps)
                    # h chunks -> hext tiles [128 j, 64] (+ ones col)
                    for c in range(b * 8, (b + 1) * 8):
                        kk, off2 = c // (XW // P), (c % (XW // P)) * P
                        h_ps = prep_ps.tile([P, DOUT], F32, tag="pp", name="h_ps")
                        nc.tensor.matmul(h_ps, lhsT=xt_t[kk][:, off2 : off2 + P],
                                         rhs=w_a, start=True, stop=False)
                        nc.tensor.matmul(h_ps, lhsT=xt_b[kk][:, off2 : off2 + P],
                                         rhs=w_b, start=False, stop=True)
                        if c % 2 == 0:
                            nc.scalar.copy(hext_c[c][:, 0:DOUT], h_ps)
                        else:
                            nc.vector.tensor_copy(hext_c[c][:, 0:DOUT], h_ps)
                        nc.vector.memset(hext_c[c][:, DOUT : DOUT + 1], 1.0)

            # src for this core's rows (dynamic SBUF slice by partition id),
            # then broadcast to all 128 partitions as fp16 (s16).
            for ib in range(2):
                s_ps = prep_ps.tile([1, 512], F32, tag="pp", name="s_ps")
                nc.tensor.matmul(
                    s_ps, lhsT=a1,
                    rhs=ht_sb[0:DOUT, bass.ds(pid * R + ib * 512, 512)],
                    start=True, stop=True)
                nc.vector.tensor_copy(src_my[:, ib * 512 : (ib + 1) * 512], s_ps)
            for ib in range(2):
                sb_ps = prep_ps.tile([P, 512], F32, tag="pp", name="sb_ps")
                nc.tensor.matmul(sb_ps, lhsT=ones1,
                                 rhs=src_my[:, ib * 512 : (ib + 1) * 512],
                                 start=True, stop=True)
                nc.vector.tensor_copy(s16[:, ib * 512 : (ib + 1) * 512], sb_ps)

        # ---- stage B: main attention loop ----
        agg_pool = ctx.enter_context(tc.tile_pool(name="agg_ps", bufs=2, space="PSUM"))
        e_pool = ctx.enter_context(tc.tile_pool(name="e_sb", bufs=E_BUFS))
        p_pool = ctx.enter_context(tc.tile_pool(name="p_sb", bufs=PBUFS))

        agg = [agg_pool.tile([65, 512], F32, tag=f"agg{ib}", name=f"agg{ib}", bufs=1)
               for ib in range(2)]

        from contextlib import nullcontext
        loop_cm = tc.For_i(0, rep_loop, 1) if rep_loop > 1 else nullcontext()
        with loop_cm:
            _stageB(nc, tc, adjm_d, e_pool, p_pool, agg, s16, dstb8,
                    alpha_t, hext_c)

        # ---- finalize ----
        with tc.tile_pool(name="fin", bufs=4) as fin, \
             tc.tile_pool(name="fin_ps", bufs=2, space="PSUM") as fin_ps:
            for ib in range(2):
                agg_sb = fin.tile([65, 512], F32, tag="agg_sb")
                nc.vector.tensor_copy(agg_sb, agg[ib])
                for q in range(4):
                    o_ps = fin_ps.tile([P, 65], F32, tag="o_ps")
                    nc.tensor.matmul(o_ps, lhsT=agg_sb[:, q * P : (q + 1) * P],
                                     rhs=idf[0:65, 0:65], start=True, stop=True)
                    rc = fin.tile([P, 1], F32, tag="rc")
                    nc.vector.reciprocal(rc, o_ps[:, DOUT : DOUT + 1])
                    hp = fin.tile([P, DOUT], F32, tag="hp")
                    nc.vector.tensor_scalar_mul(hp, o_ps[:, 0:DOUT], rc)
                    # elu = max(x,0) + exp(min(x,0)) - 1
                    ng = fin.tile([P, DOUT], F32, tag="ng")
                    nc.vector.tensor_scalar_min(ng, hp, 0.0)
                    ex = fin.tile([P, DOUT], F32, tag="ex")
                    nc.scalar.activation(ex, ng, mybir.ActivationFunctionType.Exp)
                    ps_ = fin.tile([P, DOUT], F32, tag="ps_")
                    nc.vector.tensor_scalar_max(ps_, hp, 0.0)
                    ob = fin.tile([P, DOUT], F32, tag="ob")
                    nc.vector.tensor_tensor(out=ob, in0=ex, in1=ps_,
                                            op=mybir.AluOpType.add)
                    nc.vector.tensor_scalar_add(ob, ob, -1.0)
                    g = ib * 4 + q
                    nc.sync.dma_start(out=out_d[g * P : (g + 1) * P, :], in_=ob)


def _stageB(nc, tc, adjm_d, e_pool, p_pool, agg, s16, dstb8, alpha_t, hext_c):
    pending = []

    def emit_agg(pbig_, jc_pair):
        for half, jcx in ((0, jc_pair), (1, jc_pair + 1)):
            for ib in range(2):
                nc.tensor.matmul(
                    agg[ib][:, 0:512],
                    lhsT=hext_c[jcx],
                    rhs=pbig_[:, half * 1024 + ib * 512 : half * 1024 + ib * 512 + 512],
                    start=(jcx == 0),
                    stop=(jcx == NJC - 1),
                )

    ebig = None
    for jc in range(NJC):
        half = jc % 2
        if half == 0:
            ebig = e_pool.tile([P, 2048], F16, tag="ebig")
        eb = ebig[:, half * 1024 : half * 1024 + 1024]
        dst_s = dstb8[jc // 8][:, jc % 8 : jc % 8 + 1]
        if jc % ACT_PRELU_EVERY == 0:
            nc.scalar.activation(
                eb, s16, mybir.ActivationFunctionType.Prelu,
                bias=dst_s, scale=1.0, alpha=alpha_t,
            )
        else:
            x = e_pool.tile([P, 1024], F16, tag="xls", name="xls")
            t = e_pool.tile([P, 1024], F16, tag="tls", name="tls")
            nc.vector.tensor_scalar(out=x, in0=s16, scalar1=dst_s,
                                    scalar2=None, op0=mybir.AluOpType.add)
            nc.vector.tensor_scalar(out=t, in0=x, scalar1=0.2,
                                    scalar2=None, op0=mybir.AluOpType.mult)
            nc.vector.tensor_tensor(out=eb, in0=x, in1=t,
                                    op=mybir.AluOpType.max)
        # mask: e += (adj-1)*100 via SWDGE cast int8->fp16 + CCE add
        nc.gpsimd.dma_start(
            out=eb,
            in_=adjm_d[jc * P : (jc + 1) * P, :],
            accum_op=mybir.AluOpType.add,
        )
        if half == 1:
            pbig = p_pool.tile([P, 2048], BF16, tag="pbig")
            nc.scalar.activation(pbig, ebig, mybir.ActivationFunctionType.Exp)
            pending.append((pbig, jc - 1))
            if len(pending) > AGG_DELAY:
                emit_agg(*pending.pop(0))
    while pending:
        emit_agg(*pending.pop(0))


def make_in_maps(inputs):
    xt = np.ascontiguousarray(np.asarray(inputs["input"], np.float32)[0].T)
    adj = np.asarray(inputs["adj"], np.int32)
    w = np.ascontiguousarray(np.asarray(inputs["w"], np.float32))
    a = np.ascontiguousarray(np.asarray(inputs["a"], np.float32).reshape(2 * DOUT, 1))
    # mask slab: transpose to [N, R] per core, values (adj-1)*100 in int8
    adjm = ((adj.T.astype(np.int8) - 1) * np.int8(MASKV)).astype(np.int8)
    in_maps = []
    for c in range(NCORES):
        in_maps.append({
            "adjm": np.ascontiguousarray(adjm[:, c * R : (c + 1) * R]),
            "xt": xt,
            "w": w,
            "av": a,
        })
    return in_maps


def kernel(**inputs) -> np.ndarray:
    in_maps = make_in_maps(inputs)
    nc = build_module()
    res = bass_utils.run_bass_kernel_spmd(nc, in_maps, core_ids=list(range(NCORES)))
    out = np.concatenate([res.results[c]["out"] for c in range(NCORES)], axis=0)
    return out.astype(np.float32)


if __name__ == "__main__":
    rng = np.random.default_rng(0)
    ins = {
        "input": rng.standard_normal((1, N, DIN)).astype(np.float32),
        "adj": rng.integers(0, 2, size=(N, N)).astype(np.int32),
        "w": rng.standard_normal((DIN, DOUT)).astype(np.float32) * 0.1,
        "a": rng.standard_normal((2 * DOUT, 1)).astype(np.float32) * 0.1,
    }
    o = kernel(**ins)
    print("kernel out", o.shape, o.dtype)
    # numpy reference check
    h = ins["input"][0] @ ins["w"]
    src = h @ ins["a"][:DOUT, 0]
    dst = h @ ins["a"][DOUT:, 0]
    x = src[:, None] + dst[None, :]
    e = np.where(x > 0, x, 0.2 * x)
    att = np.where(ins["adj"] > 0, e, -9e15)
    att = att - att.max(axis=1, keepdims=True)
    p = np.exp(att)
    att = p / p.sum(axis=1, keepdims=True)
    hp = att @ h
    exp_ref = np.where(hp > 0, hp, np.exp(np.minimum(hp, 0)) - 1)
    err = np.linalg.norm(o - exp_ref) / np.linalg.norm(exp_ref)
    print("rel err vs numpy:", err)


# revision 3
# speedup vs baseline: 1.1466x; 1.1466x over previous
"""GAT head (DGAT) Trainium2 kernel, v2: sorted rank-2 decomposition.

exp(leaky(s_i + d_j)) splits exactly at s_i + d_j = 0:
  p = e^{s_i} e^{d_j}           if s_i + d_j >= 0   (branch 1)
      e^{0.2 s_i} e^{0.2 d_j}   otherwise           (branch 2)

Sort j globally by d (host permutes xt columns / adjacency rows) and each
core's i-slab by s (host appends the core's 1024 columns, re-sorted, to xt
so the device computes src already in sorted order). With both axes sorted,
the branch boundary J_i = #{j : d_j < -s_i} is a monotone staircase, so

  num[i,:] = e^{s_i}     * sum_{j>=J_i} adj[j,i] e^{d_j}     [h_j|1]
           + e^{0.2 s_i} * sum_{j< J_i} adj[j,i] e^{0.2 d_j} [h_j|1]

is computed by suffix/prefix matmuls over the RAW 0/1 adjacency with
e^d-weighted hext tiles; only a narrow per-chunk crossing window needs a
runtime mask (DVE compare + mult). No per-element exp/leaky at all.
h' = num[:,0:64] / num[:,64]; out = elu(h').

The crossing-window column ranges are compile-time constants taken from the
union staircase across the 8 cores (SPMD: one module for all cores). Inside
the window the runtime mask makes any staircase exact; outside it the
classification is verified on the host at input-prep time.
"""
import os
import sys
import numpy as np

sys.path.insert(0, "/opt/trn_rl_repo")

import concourse.bass as bass
import concourse.bacc as bacc
import concourse.tile as tile
from concourse import mybir
from concourse.masks import make_identity
from concourse import bass_utils

P = 128
N = 8192
DIN = 256
DOUT = 64
NCORES = 8
R = N // NCORES          # rows per core
XN = N + R               # xt columns: N global (d-sorted) + R own (s-sorted)
ALPHA = 0.2
NJC = N // P             # 64 j-chunks
F32 = mybir.dt.float32
F16 = mybir.dt.float16
BF16 = mybir.dt.bfloat16
I8 = mybir.dt.int8

_cached = {}
ADJ_BUFS = 8
_meta = {"win": None}    # set by make_in_maps: ((lo_0,hi_0),...,(lo_63,hi_63))


def build_module(rep=1, rep_loop=1, win=None):
    if win is None:
        win = _meta["win"]
    assert win is not None, "call make_in_maps first (computes crossing windows)"
    win = tuple(tuple(x) for x in win)
    key = ("nc2", rep, rep_loop, win)
    if key in _cached:
        return _cached[key]
    nc = bacc.Bacc("TRN2", target_bir_lowering=False, debug=False, num_devices=NCORES)

    adjs_d = nc.dram_tensor("adjs", [N, R], I8, kind="ExternalInput").ap()
    x_d = nc.dram_tensor("xt", [DIN, XN], F32, kind="ExternalInput").ap()
    w_d = nc.dram_tensor("w", [DIN, DOUT], F32, kind="ExternalInput").ap()
    a_d = nc.dram_tensor("av", [2 * DOUT, 1], F32, kind="ExternalInput").ap()
    j_d = nc.dram_tensor("jthr", [1, R], F32, kind="ExternalInput").ap()
    out_d = nc.dram_tensor("out", [R, DOUT], F32, kind="ExternalOutput").ap()

    with tile.TileContext(nc) as tc:
        for _ in range(rep):
            _build(nc, tc, adjs_d, x_d, w_d, a_d, j_d, out_d, rep_loop, win)

    nc.compile()
    _cached[key] = nc
    return nc


def _build(nc, tc, adjs_d, x_d, w_d, a_d, j_d, out_d, rep_loop, win):
    from contextlib import ExitStack, nullcontext

    with ExitStack() as ctx:
        const = ctx.enter_context(tc.tile_pool(name="const", bufs=1))

        # ---- constants ----
        idf = const.tile([P, P], F32)
        make_identity(nc, idf)
        ones1 = const.tile([1, P], F32)
        nc.vector.memset(ones1, 1.0)
        zrhs = const.tile([1, 512], F32)
        nc.vector.memset(zrhs, 0.0)

        w_a = const.tile([P, DOUT], F32)
        w_b = const.tile([P, DOUT], F32)
        nc.sync.dma_start(out=w_a, in_=w_d[0:P, :])
        nc.sync.dma_start(out=w_b, in_=w_d[P:DIN, :])
        a1 = const.tile([DOUT, 1], F32)
        a2 = const.tile([DOUT, 1], F32)
        nc.sync.dma_start(out=a1, in_=a_d[0:DOUT, :])
        nc.sync.dma_start(out=a2, in_=a_d[DOUT : 2 * DOUT, :])
        jrow = const.tile([1, R], F32)
        nc.sync.dma_start(out=jrow, in_=j_d)

        # persistent per-core data
        hx1 = [const.tile([P, 65], BF16, tag=f"h1{c}", name=f"h1{c}")
               for c in range(NJC)]             # e^{d_j} * [h|1]
        hx2 = [const.tile([P, 65], BF16, tag=f"h2{c}", name=f"h2{c}")
               for c in range(NJC)]             # e^{0.2 d_j} * [h|1]
        hx2n = [const.tile([P, 65], BF16, tag=f"h2n{c}", name=f"h2n{c}")
                for c in range(NJC)]            # -e^{0.2 d_j} * [h|1]
        jb = const.tile([P, R], F32)            # J_i broadcast to 128 parts
        jgall = const.tile([P, NJC], F32)       # jgall[p,c] = 128c + p
        src_my = const.tile([1, R], F32)        # s, sorted ascending
        es = const.tile([1, R], F32)            # e^{s}
        es02 = const.tile([1, R], F32)          # e^{0.2 s}

        # ---- stage A ----
        XCH = 2
        XW = XN // XCH           # 4608 (multiple of 512)
        with tc.tile_pool(name="prep", bufs=1) as prep, \
             tc.tile_pool(name="prep_ps", bufs=2, space="PSUM") as prep_ps:
            xt_t = [prep.tile([P, XW], F32, tag=f"xta{k}", name=f"xta{k}")
                    for k in range(XCH)]
            xt_b = [prep.tile([P, XW], F32, tag=f"xtb{k}", name=f"xtb{k}")
                    for k in range(XCH)]
            ht_sb = prep.tile([DOUT, XN], F32)   # h^T (cols: N sorted | R own)
            ed8 = [prep.tile([P, 8], F32, tag=f"e1{b}", name=f"e1{b}")
                   for b in range(NJC // 8)]
            ed028 = [prep.tile([P, 8], F32, tag=f"e2{b}", name=f"e2{b}")
                     for b in range(NJC // 8)]
            hxc = prep.tile([P, 65], BF16, tag="hxc", name="hxc")

            for k in range(XCH):
                nc.sync.dma_start(out=xt_t[k], in_=x_d[0:P, k * XW : (k + 1) * XW])
                nc.sync.dma_start(out=xt_b[k], in_=x_d[P:DIN, k * XW : (k + 1) * XW])

            # iota over partitions -> jgall[p, c] = 128c + p
            iot = prep.tile([P, 1], mybir.dt.int32, tag="iot", name="iot")
            nc.gpsimd.iota(iot, pattern=[[0, 1]], base=0, channel_multiplier=1)
            nc.vector.tensor_copy(jgall[:, 0:1], iot)
            for c in range(1, NJC):
                nc.vector.tensor_scalar_add(jgall[:, c : c + 1], jgall[:, 0:1],
                                            float(128 * c))

            # hT [64, XN] via f32r
            nmm = XN // 512      # 18
            for m in range(nmm):
                k, off = (m * 512) // XW, (m * 512) % XW
                ht_ps = prep_ps.tile([DOUT, 512], F32, tag="pp", name="ht_ps")
                nc.tensor.matmul(ht_ps, lhsT=w_a,
                                 rhs=xt_t[k][:, off : off + 512],
                                 start=True, stop=False)
                nc.tensor.matmul(ht_ps, lhsT=w_b,
                                 rhs=xt_b[k][:, off : off + 512],
                                 start=False, stop=True)
                if m % 2 == 0:
                    nc.vector.tensor_copy(ht_sb[:, m * 512 : (m + 1) * 512], ht_ps)
                else:
                    nc.scalar.copy(ht_sb[:, m * 512 : (m + 1) * 512], ht_ps)

            # dst batches -> ed8/ed028 (exp), then weighted hext tiles
            for b in range(NJC // 8):
                d_ps = prep_ps.tile([P, 8], F32, tag="pp", name="d_ps")
                for bb in range(8):
                    c = b * 8 + bb
                    nc.tensor.matmul(d_ps[:, bb : bb + 1],
                                     lhsT=ht_sb[:, c * P : (c + 1) * P], rhs=a2,
                                     start=True, stop=True)
                nc.scalar.activation(ed8[b], d_ps,
                                     mybir.ActivationFunctionType.Exp)
                nc.scalar.activation(ed028[b], d_ps,
                                     mybir.ActivationFunctionType.Exp, scale=0.2)
                for c in range(b * 8, (b + 1) * 8):
                    kk, off2 = (c * P) // XW, (c * P) % XW
                    h_ps = prep_ps.tile([P, DOUT], F32, tag="pp", name="h_ps")
                    nc.tensor.matmul(h_ps, lhsT=xt_t[kk][:, off2 : off2 + P],
                                     rhs=w_a, start=True, stop=False)
                    nc.tensor.matmul(h_ps, lhsT=xt_b[kk][:, off2 : off2 + P],
                                     rhs=w_b, start=False, stop=True)
                    if c % 2 == 0:
                        nc.scalar.copy(hxc[:, 0:DOUT], h_ps)
                    else:
                        nc.vector.tensor_copy(hxc[:, 0:DOUT], h_ps)
                    nc.vector.memset(hxc[:, DOUT : DOUT + 1], 1.0)
                    e1 = ed8[b][:, c - b * 8 : c - b * 8 + 1]
                    e2 = ed028[b][:, c - b * 8 : c - b * 8 + 1]
                    nc.vector.tensor_scalar(out=hx1[c], in0=hxc, scalar1=e1,
                                            scalar2=None, op0=mybir.AluOpType.mult)
                    nc.vector.tensor_scalar(out=hx2[c], in0=hxc, scalar1=e2,
                                            scalar2=None, op0=mybir.AluOpType.mult)
                    nc.vector.tensor_scalar(out=hx2n[c], in0=hx2[c], scalar1=-1.0,
                                            scalar2=None, op0=mybir.AluOpType.mult)

            # src (own block, already sorted): cols [N, N+R)
            for ib in range(2):
                s_ps = prep_ps.tile([1, 512], F32, tag="pp", name="s_ps")
                nc.tensor.matmul(
                    s_ps, lhsT=a1,
                    rhs=ht_sb[0:DOUT, N + ib * 512 : N + (ib + 1) * 512],
                    start=True, stop=True)
                nc.vector.tensor_copy(src_my[:, ib * 512 : (ib + 1) * 512], s_ps)
            nc.scalar.activation(es, src_my, mybir.ActivationFunctionType.Exp)
            nc.scalar.activation(es02, src_my, mybir.ActivationFunctionType.Exp,
                                 scale=0.2)
            # J broadcast to 128 partitions
            for ib in range(2):
                jb_ps = prep_ps.tile([P, 512], F32, tag="pp", name="jb_ps")
                nc.tensor.matmul(jb_ps, lhsT=ones1,
                                 rhs=jrow[:, ib * 512 : (ib + 1) * 512],
                                 start=True, stop=True)
                nc.vector.tensor_copy(jb[:, ib * 512 : (ib + 1) * 512], jb_ps)

        # ---- stage B: staircase aggregation ----
        agg_pool = ctx.enter_context(tc.tile_pool(name="agg_ps", bufs=1, space="PSUM"))
        adj_pool = ctx.enter_context(tc.tile_pool(name="adjs", bufs=ADJ_BUFS))
        m_pool = ctx.enter_context(tc.tile_pool(name="msk", bufs=4))

        out1 = agg_pool.tile([65, R], F32, tag="out1", name="out1")
        out2 = agg_pool.tile([65, R], F32, tag="out2", name="out2")

        loop_cm = tc.For_i(0, rep_loop, 1) if rep_loop > 1 else nullcontext()
        with loop_cm:
            _stageB(nc, tc, adjs_d, adj_pool, m_pool, out1, out2,
                    hx1, hx2, hx2n, jb, jgall, ones1, zrhs, win)

        # ---- finalize: num = es*out1 + es02*out2, transpose, ELU ----
        with tc.tile_pool(name="fin", bufs=4) as fin, \
             tc.tile_pool(name="fin_ps", bufs=2, space="PSUM") as fin_ps:
            o1s = fin.tile([65, R], F32, tag="o1s", name="o1s")
            o2s = fin.tile([65, R], F32, tag="o2s", name="o2s")
            nc.vector.tensor_copy(o1s, out1)
            nc.vector.tensor_copy(o2s, out2)
            esb = fin.tile([65, R], F32, tag="esb", name="esb")
            esb2 = fin.tile([65, R], F32, tag="esb2", name="esb2")
            for ib in range(2):
                e_ps = fin_ps.tile([65, 512], F32, tag="fp", name="e_ps")
                nc.tensor.matmul(e_ps, lhsT=ones1[:, 0:65],
                                 rhs=es[:, ib * 512 : (ib + 1) * 512],
                                 start=True, stop=True)
                nc.vector.tensor_copy(esb[:, ib * 512 : (ib + 1) * 512], e_ps)
                e_ps2 = fin_ps.tile([65, 512], F32, tag="fp", name="e_ps2")
                nc.tensor.matmul(e_ps2, lhsT=ones1[:, 0:65],
                                 rhs=es02[:, ib * 512 : (ib + 1) * 512],
                                 start=True, stop=True)
                nc.vector.tensor_copy(esb2[:, ib * 512 : (ib + 1) * 512], e_ps2)
            num = fin.tile([65, R], F32, tag="num", name="num")
            nc.vector.tensor_tensor(out=num, in0=o1s, in1=esb,
                                    op=mybir.AluOpType.mult)
            nc.vector.tensor_tensor(out=o2s, in0=o2s, in1=esb2,
                                    op=mybir.AluOpType.mult)
            nc.vector.tensor_tensor(out=num, in0=num, in1=o2s,
                                    op=mybir.AluOpType.add)
            for q in range(8):
                o_ps = fin_ps.tile([P, 65], F32, tag="fo", name="o_ps")
                nc.tensor.matmul(o_ps, lhsT=num[:, q * P : (q + 1) * P],
                                 rhs=idf[0:65, 0:65], start=True, stop=True)
                rc = fin.tile([P, 1], F32, tag="rc")
                nc.vector.reciprocal(rc, o_ps[:, DOUT : DOUT + 1])
                hp = fin.tile([P, DOUT], F32, tag="hp")
                nc.vector.tensor_scalar_mul(hp, o_ps[:, 0:DOUT], rc)
                ng = fin.tile([P, DOUT], F32, tag="ng")
                nc.vector.tensor_scalar_min(ng, hp, 0.0)
                ex = fin.tile([P, DOUT], F32, tag="ex")
                nc.scalar.activation(ex, ng, mybir.ActivationFunctionType.Exp)
                ps_ = fin.tile([P, DOUT], F32, tag="ps_")
                nc.vector.tensor_scalar_max(ps_, hp, 0.0)
                ob = fin.tile([P, DOUT], F32, tag="ob")
                nc.vector.tensor_tensor(out=ob, in0=ex, in1=ps_,
                                        op=mybir.AluOpType.add)
                nc.vector.tensor_scalar_add(ob, ob, -1.0)
                nc.sync.dma_start(out=out_d[q * P : (q + 1) * P, :], in_=ob)


def _ranged_mm(nc, out_ps, lhsT, rhs_tile, lo, hi, **kw):
    """matmul into out_ps[:, lo:hi] split at the 512 psum-bank boundary."""
    for a, b in ((lo, min(hi, 512)), (max(lo, 512), hi)):
        if b > a:
            nc.tensor.matmul(out_ps[:, a:b], lhsT=lhsT, rhs=rhs_tile[:, a:b],
                             skip_group_check=True, **kw)


def _stageB(nc, tc, adjs_d, adj_pool, m_pool, out1, out2,
            hx1, hx2, hx2n, jb, jgall, ones1, zrhs, win):
    # zero-init psum accumulation regions
    for ps in (out1, out2):
        for ib in range(2):
            nc.tensor.matmul(ps[:, ib * 512 : (ib + 1) * 512],
                             lhsT=ones1[:, 0:65], rhs=zrhs,
                             start=True, stop=False, skip_group_check=True)
    mm = dict(start=False, stop=False)
    for c in range(NJC):
        lo, hi = win[c]
        at = adj_pool.tile([P, R], BF16, tag="adjs")
        nc.gpsimd.dma_start(out=at, in_=adjs_d[c * P : (c + 1) * P, :])
        # full columns: [hi, R) all rows >= J_i -> out1; [0, lo) -> out2
        _ranged_mm(nc, out1, hx1[c], at, hi, R, **mm)
        _ranged_mm(nc, out2, hx2[c], at, 0, lo, **mm)
        if hi > lo:
            # crossing window: mask m = [J_i <= jg], am1 = m * adj
            wmax = max(h - l for l, h in win)
            mfull = m_pool.tile([P, wmax], BF16, tag="m", name="m")
            m = mfull[:, 0 : hi - lo]
            nc.vector.tensor_scalar(out=m, in0=jb[:, lo:hi],
                                    scalar1=jgall[:, c : c + 1], scalar2=None,
                                    op0=mybir.AluOpType.is_le)
            amfull = m_pool.tile([P, wmax], BF16, tag="am", name="am")
            am = amfull[:, 0 : hi - lo]
            nc.vector.tensor_tensor(out=am, in0=m, in1=at[:, lo:hi],
                                    op=mybir.AluOpType.mult)
            # out1 += hx1^T @ am ; out2 += hx2^T @ (adj - am)
            for a, b in ((lo, min(hi, 512)), (max(lo, 512), hi)):
                if b > a:
                    nc.tensor.matmul(out1[:, a:b], lhsT=hx1[c],
                                     rhs=am[:, a - lo : b - lo],
                                     skip_group_check=True, **mm)
                    nc.tensor.matmul(out2[:, a:b], lhsT=hx2[c],
                                     rhs=at[:, a:b],
                                     skip_group_check=True, **mm)
                    nc.tensor.matmul(out2[:, a:b], lhsT=hx2n[c],
                                     rhs=am[:, a - lo : b - lo],
                                     skip_group_check=True, **mm)
    # close accumulation groups
    for ps in (out1, out2):
        for ib in range(2):
            nc.tensor.matmul(ps[:, ib * 512 : (ib + 1) * 512],
                             lhsT=ones1[:, 0:65], rhs=zrhs,
                             start=False, stop=True, skip_group_check=True)


def make_in_maps(inputs):
    x = np.asarray(inputs["input"], np.float32)[0]        # [N, DIN]
    adj = np.asarray(inputs["adj"], np.int32)
    w = np.ascontiguousarray(np.asarray(inputs["w"], np.float32))
    a = np.asarray(inputs["a"], np.float32).reshape(2 * DOUT, 1)

    h = x @ w
    src = h @ a[:DOUT, 0]
    dst = h @ a[DOUT:, 0]
    pi = np.argsort(dst, kind="stable")                   # global j-order
    d_sorted = dst[pi]
    adjP = adj[:, pi]                                     # cols -> j sorted

    in_maps = []
    row_order = []
    wins = []
    for c in range(NCORES):
        blk = pi[c * R : (c + 1) * R]                     # this core's nodes
        rho = np.argsort(src[blk], kind="stable")
        rows = blk[rho]                                   # i, s-sorted
        row_order.append(rows)
        jthr = np.searchsorted(d_sorted, -src[rows], side="left")
        # staircase sanity: J non-increasing along sorted i
        assert np.all(np.diff(jthr) <= 0)
        xt_c = np.concatenate([x[pi].T, x[rows].T], axis=1)  # [DIN, XN]
        adjs = adjP[rows].T                               # [N j-sorted, R]
        in_maps.append({
            "adjs": np.ascontiguousarray(adjs.astype(np.int8)),
            "xt": np.ascontiguousarray(xt_c),
            "w": w,
            "av": np.ascontiguousarray(a),
            "jthr": np.ascontiguousarray(jthr.astype(np.float32)[None, :]),
        })
        # per-chunk crossing windows for this core
        wc = []
        for cc in range(NJC):
            # cols with 128c < J_i < 128(c+1) need masking; J desc in i
            hi1 = int(np.searchsorted(-jthr, -(128 * cc), side="left"))
            lo2 = int(np.searchsorted(-jthr, -(128 * (cc + 1) - 1), side="left"))
            wc.append((lo2, hi1))
        wins.append(wc)
    # union window across cores (module is SPMD-shared)
    win = tuple((min(wins[k][cc][0] for k in range(NCORES)),
                 max(wins[k][cc][1] for k in range(NCORES)))
                for cc in range(NJC))
    # verify coverage: outside the window the full-tile classification holds
    for k in range(NCORES):
        jthr = in_maps[k]["jthr"][0].astype(np.int64)
        for cc in range(NJC):
            lo, hi = win[cc]
            if hi < R:
                assert np.all(jthr[hi:] <= 128 * cc)
            if lo > 0:
                assert np.all(jthr[:lo] >= 128 * (cc + 1))
    _meta["win"] = win
    _meta["row_order"] = np.concatenate(row_order)
    return in_maps


def kernel(**inputs) -> np.ndarray:
    in_maps = make_in_maps(inputs)
    nc = build_module()
    res = bass_utils.run_bass_kernel_spmd(nc, in_maps, core_ids=list(range(NCORES)))
    out_sorted = np.concatenate([res.results[c]["out"] for c in range(NCORES)],
                                axis=0)
    out = np.empty_like(out_sorted)
    out[_meta["row_order"]] = out_sorted
    return out.astype(np.float32)


if __name__ == "__main__":
    rng = np.random.default_rng(0)
    ins = {
        "input": rng.standard_normal((1, N, DIN)).astype(np.float32),
        "adj": rng.integers(0, 2, size=(N, N)).astype(np.int32),
        "w": rng.standard_normal((DIN, DOUT)).astype(np.float32) * 0.1,
        "a": rng.standard_normal((2 * DOUT, 1)).astype(np.float32) * 0.1,
    }
    o = kernel(**ins)
    print("kernel out", o.shape, o.dtype)
    h = ins["input"][0] @ ins["w"]
    src = h @ ins["a"][:DOUT, 0]
    dst = h @ ins["a"][DOUT:, 0]
    xx = src[:, None] + dst[None, :]
    e = np.where(xx > 0, xx, 0.2 * xx)
    att = np.where(ins["adj"] > 0, e, -9e15)
    att = att - att.max(axis=1, keepdims=True)
    p = np.exp(att)
    att = p / p.sum(axis=1, keepdims=True)
    hp = att @ h
    exp_ref = np.where(hp > 0, hp, np.exp(np.minimum(hp, 0)) - 1)
    err = np.linalg.norm(o - exp_ref) / np.linalg.norm(exp_ref)
    print("rel err vs numpy:", err)
    w_ = np.array(_meta["win"])
    print("window widths: mean", (w_[:, 1] - w_[:, 0]).mean(),
          "max", (w_[:, 1] - w_[:, 0]).max())
